# revision 8
# baseline (speedup 1.0000x reference)
"""HARCNet Trainium2 kernel: capsule attention net, data-parallel over 8 cores.

Pipeline per core (8192 rows = 16 groups x 512):
  DMA x -> PE-transpose x to feature-major -> cap matmul (561-contraction)
  -> replicated Q/K/V projection matmuls (biases folded algebraically)
  -> DVE muls + PE partition-group reductions for the 4x4 capsule attention
  -> exp/softmax (no max-sub; scores are O(1)) -> MLP head -> PE-transpose out
All weight tensors are tiny and precomputed host-side into fused forms.
"""

import os

import numpy as np

import concourse.bass as bass
import concourse.mybir as mybir
import concourse.tile as tile
from concourse import masks
from concourse.bass_utils import run_bass_kernel_spmd

F32 = mybir.dt.float32
F32R = mybir.dt.float32r
AFT = mybir.ActivationFunctionType

N_CORES = 8
B_FULL = 65536
D_IN = 561
B_CORE = B_FULL // N_CORES      # 8192
GROUP = 512
N_GROUPS = B_CORE // GROUP      # 16
NSUB = GROUP // 128             # 4
K_CHUNKS = [(0, 128), (128, 128), (256, 128), (384, 128), (512, 49)]

# float32r runs the PE at 4x the fp32 rate; flip after verifying accuracy on HW.
USE_F32R_MM = os.environ.get("HARC_F32R_MM", "1") == "1"
USE_F32R_TR = os.environ.get("HARC_F32R_TR", "0") == "1"
# Loop the whole pipeline R times on-device (timing experiments only).
REPEATS = int(os.environ.get("HARC_REPEATS", "1"))


def _prep_weights(inp):
    """Fuse all the tiny parameters host-side (float64 -> float32)."""
    f64 = lambda a: np.asarray(a, np.float64)
    fw = f64(inp["feature_weights"])
    cap_W, cap_b = f64(inp["cap_W"]), f64(inp["cap_b"])
    q_W, q_b = f64(inp["q_W"]), f64(inp["q_b"])
    k_W, k_b = f64(inp["k_W"]), f64(inp["k_b"])
    v_W, v_b = f64(inp["v_W"]), f64(inp["v_b"])
    fc1_W, fc1_b = f64(inp["fc1_W"]), f64(inp["fc1_b"])
    fc2_W, fc2_b = f64(inp["fc2_W"]), f64(inp["fc2_b"])
    res_W, res_b = f64(inp["res_W"]), f64(inp["res_b"])
    out_W, out_b = f64(inp["out_W"]), f64(inp["out_b"])

    W1 = cap_W * fw[None, :]                      # [32, 561]
    capb = cap_b.reshape(4, 8)

    # pi = (q, k, d) = q*32 + k*8 + d ; c = (i, e) = i*8 + e
    WQ = np.zeros((4, 4, 8, 4, 8))
    WK = np.zeros((4, 4, 8, 4, 8))
    WV = np.zeros((4, 4, 8, 4, 8))
    for q in range(4):
        for k in range(4):
            WQ[q, k, :, q, :] = q_W
            WK[q, k, :, k, :] = k_W
            WV[q, k, :, k, :] = v_W
    WQ = WQ.reshape(128, 32)
    WK = WK.reshape(128, 32)
    WV = WV.reshape(128, 32)

    cq = np.zeros((4, 4, 8))
    ck = np.zeros((4, 4, 8))
    cv = np.zeros((4, 4, 8))
    for q in range(4):
        for k in range(4):
            cq[q, k, :] = q_W @ capb[q] + q_b
            ck[q, k, :] = k_W @ capb[k] + k_b
            cv[q, k, :] = v_W @ capb[k] + v_b

    s8 = 1.0 / np.sqrt(8.0)
    # scores = R @ (Qraw*Kraw) + M1 @ capsT + c0
    R = np.zeros((16, 128))
    for q in range(4):
        for k in range(4):
            s = q * 4 + k
            R[s, q * 32 + k * 8: q * 32 + k * 8 + 8] = s8
    M1 = np.zeros((16, 32))
    c0 = np.zeros(16)
    cq_f = cq.reshape(128)
    ck_f = ck.reshape(128)
    for q in range(4):
        for k in range(4):
            s = q * 4 + k
            sl = slice(q * 32 + k * 8, q * 32 + k * 8 + 8)
            M1[s, :] = s8 * (cq_f[sl] @ WK[sl, :] + ck_f[sl] @ WQ[sl, :])
            c0[s] = s8 * np.dot(cq_f[sl], ck_f[sl])

    REP = np.zeros((128, 16))
    for q in range(4):
        for k in range(4):
            REP[q * 32 + k * 8: q * 32 + k * 8 + 8, q * 4 + k] = 1.0
    DN = np.zeros((32, 16))
    for q in range(4):
        for k in range(4):
            DN[q * 8: q * 8 + 8, q * 4 + k] = 1.0
    AR = np.zeros((32, 128))
    MV = np.zeros((32, 16))
    cv3 = cv  # [q, k, d]
    for q in range(4):
        for d in range(8):
            for k in range(4):
                AR[q * 8 + d, q * 32 + k * 8 + d] = 1.0
                MV[q * 8 + d, q * 4 + k] = cv3[q, k, d]

    f32 = lambda a: np.ascontiguousarray(a, np.float32)
    return {
        "w1t": f32(W1.T),                       # [561, 32]
        "wqt": f32(WQ.T),                       # [32, 128]
        "wkt": f32(WK.T),
        "wvt": f32(WV.T),
        "rt": f32(R.T),                         # [128, 16]
        "m1t": f32(M1.T),                       # [32, 16]
        "c0": f32(c0.reshape(16, 1)),
        "rept": f32(REP.T),                     # [16, 128]
        "dnt": f32(DN.T),                       # [16, 32]
        "art": f32(AR.T),                       # [128, 32]
        "mvt": f32(MV.T),                       # [16, 32]
        "fc1t": f32(fc1_W.T),                   # [32, 64]
        "b1": f32(fc1_b.reshape(64, 1)),
        "fc2t": f32(fc2_W.T),                   # [64, 64]
        "rst": f32(res_W.T),                    # [32, 64]
        "b2": f32((fc2_b + res_b).reshape(64, 1)),
        "owt": f32(out_W.T),                    # [64, 6]
        "ob": f32(out_b.reshape(6, 1)),
    }


BIAS_NAMES = ("c0", "b1", "b2", "ob")


def _split_matmul_waits(nc):
    """Several TRN2 ISA structs (LDWEIGHTS of self-loading fp32/f32r matmuls,
    HWDGE DMA triggers) hold a single sync-wait; walrus rejects instructions
    with more. Move extra waits onto same-queue NoOps placed right before the
    instruction — identical semantics, the queue stalls on the nops first."""
    for f in nc.m.functions:
        for blk in f.blocks:
            new = []
            for inst in blk.instructions:
                si = getattr(inst, "sync_info", None)
                if si is not None and si.on_wait and len(si.on_wait) > 1:
                    waits = list(si.on_wait)
                    for w in waits[:-1]:
                        nop = mybir.InstNoOp(
                            name=nc.get_next_instruction_name(),
                            engine=inst.engine, ins=[], outs=[])
                        nop.sync_info = mybir.SyncInfo(on_wait=[w], on_update=[])
                        new.append(nop)
                    inst.sync_info = mybir.SyncInfo(
                        on_wait=[waits[-1]], on_update=si.on_update)
                new.append(inst)
            blk.instructions[:] = new


def _build_nc(wshapes):
    nc = bass.Bass("TRN2", target_bir_lowering=False, debug=False)
    mm_dt = F32R if USE_F32R_MM else F32
    tr_dt = F32R if USE_F32R_TR else F32

    x_d = nc.dram_tensor("x", [B_CORE, D_IN], F32, kind="ExternalInput")
    y_d = nc.dram_tensor("y", [B_CORE, 6], F32, kind="ExternalOutput")
    w_d = {
        n: nc.dram_tensor(n, list(s), F32 if n in BIAS_NAMES else mm_dt,
                          kind="ExternalInput")
        for n, s in wshapes.items()
    }

    def mm(out, lhsT, rhs, start, stop):
        nc.tensor.matmul(out, lhsT, rhs, start=start, stop=stop)

    with tile.TileContext(nc, trace_sim=False) as tc:
        with (
            tc.tile_pool(name="consts", bufs=1) as cpool,
            tc.tile_pool(name="xnat", bufs=8) as xnat_pool,
            tc.tile_pool(name="xtp", bufs=10) as xt_pool,
            tc.tile_pool(name="work", bufs=3) as work,
            tc.tile_pool(name="psxt", bufs=3, space="PSUM") as ps_xt_pool,
            tc.tile_pool(name="psmm", bufs=5, space="PSUM") as ps_mm,
        ):
            ident = cpool.tile([128, 128], tr_dt)
            masks.make_identity(nc, ident[:])

            w_sb = {}
            for n, s in wshapes.items():
                if n == "w1t":
                    continue
                t = cpool.tile(list(s), F32 if n in BIAS_NAMES else mm_dt, tag=n)
                nc.sync.dma_start(t[:], w_d[n][:])
                w_sb[n] = t
            w1k = []
            for lo, sz in K_CHUNKS:
                t = cpool.tile([sz, 32], mm_dt, tag=f"w1k{lo}")
                nc.sync.dma_start(t[:], w_d["w1t"][lo:lo + sz, :])
                w1k.append(t)

            for g in range(N_GROUPS):
                b0 = g * GROUP
                xns = []
                for j in range(NSUB):
                    xn = xnat_pool.tile([128, D_IN], tr_dt, tag="xn")
                    nc.sync.dma_start(
                        xn[:], x_d[b0 + j * 128: b0 + (j + 1) * 128, :].bitcast(tr_dt))
                    # noqa
                    xns.append(xn)

                xts = []
                for k, (lo, sz) in enumerate(K_CHUNKS):
                    pxt = ps_xt_pool.tile([128, GROUP], F32, tag="xt")
                    for j in range(NSUB):
                        nc.tensor.transpose(
                            pxt[:sz, j * 128:(j + 1) * 128].bitcast(tr_dt),
                            xns[j][:, lo:lo + sz],
                            ident[:, :128])
                    st = xt_pool.tile([128, GROUP], mm_dt, tag="xts")
                    if k % 2 == 0:
                        nc.scalar.copy(st[:sz, :], pxt[:sz, :])
                    else:
                        nc.vector.tensor_copy(st[:sz, :], pxt[:sz, :])
                    xts.append(st)

                pcaps = ps_mm.tile([32, GROUP], F32, tag="mm")
                for k, (lo, sz) in enumerate(K_CHUNKS):
                    mm(pcaps[:], w1k[k][:], xts[k][:sz, :],
                       start=(k == 0), stop=(k == len(K_CHUNKS) - 1))
                caps_sb = work.tile([32, GROUP], mm_dt, tag="caps")
                nc.scalar.copy(caps_sb[:], pcaps[:])

                pq = ps_mm.tile([128, GROUP], F32, tag="mm")
                mm(pq[:], w_sb["wqt"][:], caps_sb[:], True, True)
                pk = ps_mm.tile([128, GROUP], F32, tag="mm")
                mm(pk[:], w_sb["wkt"][:], caps_sb[:], True, True)
                pv = ps_mm.tile([128, GROUP], F32, tag="mm")
                mm(pv[:], w_sb["wvt"][:], caps_sb[:], True, True)

                k_sb = work.tile([128, GROUP], mm_dt, tag="ksb")
                nc.scalar.copy(k_sb[:], pk[:])
                v_sb = work.tile([128, GROUP], mm_dt, tag="vsb")
                nc.scalar.copy(v_sb[:], pv[:])

                prod = work.tile([128, GROUP], mm_dt, tag="prod")
                nc.vector.tensor_mul(prod[:], pq[:], k_sb[:].bitcast(F32))

                ps = ps_mm.tile([16, GROUP], F32, tag="mm")
                mm(ps[:], w_sb["rt"][:], prod[:], True, False)
                mm(ps[:], w_sb["m1t"][:], caps_sb[:], False, True)

                e_sb = work.tile([16, GROUP], mm_dt, tag="esb")
                nc.scalar.activation(e_sb[:], ps[:], AFT.Exp,
                                     bias=w_sb["c0"][:, 0:1])

                per = ps_mm.tile([128, GROUP], F32, tag="mm")
                mm(per[:], w_sb["rept"][:], e_sb[:], True, True)
                pdn = ps_mm.tile([32, GROUP], F32, tag="mm")
                mm(pdn[:], w_sb["dnt"][:], e_sb[:], True, True)

                prod2 = work.tile([128, GROUP], mm_dt, tag="prod2")
                nc.vector.tensor_mul(prod2[:], per[:], v_sb[:].bitcast(F32))

                pau = ps_mm.tile([32, GROUP], F32, tag="mm")
                mm(pau[:], w_sb["art"][:], prod2[:], True, False)
                mm(pau[:], w_sb["mvt"][:], e_sb[:], False, True)

                recip = work.tile([32, GROUP], F32, tag="recip")
                nc.vector.reciprocal(recip[:], pdn[:])
                flat = work.tile([32, GROUP], mm_dt, tag="flat")
                nc.vector.tensor_mul(flat[:], pau[:], recip[:])

                ph1 = ps_mm.tile([64, GROUP], F32, tag="mm")
                mm(ph1[:], w_sb["fc1t"][:], flat[:], True, True)
                h1 = work.tile([64, GROUP], mm_dt, tag="h1")
                nc.scalar.activation(h1[:], ph1[:], AFT.Relu,
                                     bias=w_sb["b1"][:, 0:1])

                ph2 = ps_mm.tile([64, GROUP], F32, tag="mm")
                mm(ph2[:], w_sb["fc2t"][:], h1[:], True, False)
                mm(ph2[:], w_sb["rst"][:], flat[:], False, True)
                h2 = work.tile([64, GROUP], mm_dt, tag="h2")
                nc.scalar.activation(h2[:], ph2[:], AFT.Relu,
                                     bias=w_sb["b2"][:, 0:1])

                po = ps_mm.tile([6, GROUP], F32, tag="mm")
                mm(po[:], w_sb["owt"][:], h2[:], True, True)
                o_sb = work.tile([6, GROUP], F32, tag="osb")
                nc.vector.tensor_scalar_add(o_sb[:], po[:], w_sb["ob"][:, 0:1])

                ponat = ps_mm.tile([128, NSUB * 6], F32, tag="mm")
                for j in range(NSUB):
                    nc.tensor.transpose(
                        ponat[:, j * 6:(j + 1) * 6],
                        o_sb[:, j * 128:(j + 1) * 128],
                        ident.bitcast(F32)[:6, :6])
                onat = work.tile([128, NSUB * 6], F32, tag="onat")
                nc.vector.tensor_copy(onat[:], ponat[:])

                yv = y_d[b0:b0 + GROUP, :].rearrange("(j p) c -> p j c", p=128)
                nc.sync.dma_start(yv, onat[:].rearrange("p (j c) -> p j c", j=NSUB))

    _split_matmul_waits(nc)
    return nc


_CACHE = {}


def _get_nc(wshapes):
    key = tuple(sorted(wshapes.items()))
    if key not in _CACHE:
        _CACHE[key] = _build_nc(wshapes)
    return _CACHE[key]


def run(trace=False, **inputs):
    w = _prep_weights(inputs)
    x = np.ascontiguousarray(np.asarray(inputs["x"], np.float32))
    assert x.shape == (B_FULL, D_IN)
    nc = _get_nc({n: a.shape for n, a in w.items()})
    in_maps = []
    for c in range(N_CORES):
        m = {"x": np.ascontiguousarray(x[c * B_CORE:(c + 1) * B_CORE])}
        m.update(w)
        in_maps.append(m)
    res = run_bass_kernel_spmd(nc, in_maps, list(range(N_CORES)), trace=trace)
    y = np.concatenate([np.asarray(res.results[c]["y"]) for c in range(N_CORES)], axis=0)
    return y.astype(np.float32), res


def kernel(**inputs):
    y, _ = run(trace=False, **inputs)
    return y
